# revision 13
# baseline (speedup 1.0000x reference)
"""Masked phase-locking value (PLV) kernel for Trainium2, 8 NeuronCores.

Math: out[b] = |sum_ij M_ij * exp(i*(a_bi - b_bj))| / max(sum(M), 1)
    real_b = ca_b^T M cb_b + sa_b^T M sb_b
    imag_b = sa_b^T M cb_b - ca_b^T M sb_b

Device decomposition (per core, Na sharded 8 ways -> 1024 rows each),
*transposed* orientation so the j-contraction (Nb = 8192) runs on the PE:

    Z[m, i] = sum_j CS[j, m] * maskT[j, i]      (TensorE; CS = [cb^T | sb^T],
                                                 m = 2B = 128, i = 1024)
    racc[m] = sum_i Z[m, i] * WR[m, i]          (DVE scalar_tensor_tensor)
    qacc[m] = sum_i Z[m, i] * WI[m, i]          (GpSimd; WR = [ca|sa], WI = [sa|-ca])

real_b = sum_cores racc[b] + racc[64+b]; imag_b = qacc[b] + qacc[64+b].
vs the j-reduce-on-DVE orientation this shrinks the epilogue 8x; the PE does
the big reduction.

The j-accumulation is split in two PSUM tiles (jc 0-31 -> Za, 32-63 -> Zb) so
the first epilogue half overlaps the second half's matmuls.

DMA: mask groups alternate between the two HWDGE rings (sync + scalar) so the
per-ring dma_start kick (~0.6 us DIRECT2D) and inter-group gaps overlap
between rings; cs (fp8) pieces lead each ring. PE warm-up runs from a memset
tile so it needs no DMA and beats the HAM cold clock during the DMA lead-in.
dtypes: mask 0/1 fp8e4 (exact); cs fp8e4 (b-side quantization noise is
incoherent, ~3e-4 of the coherent real part); wr fp16; PSUM/epilogue fp32.
"""

import numpy as np

import concourse.bass as bass
import concourse.tile as tile
from concourse import bacc, mybir
from concourse.bass_utils import run_bass_kernel_spmd

B = 64
NA = 8192
NB = 8192
NCORES = 8
NISH = NA // NCORES          # mask rows (i) per core
JCH = NB // 128              # j contraction chunks of 128

# mask DMA groups in jc units; ring alternates per group. Small first groups
# start the PE early; small last groups shorten the post-stream tail.
# A boundary must land exactly at HALF (32) for the Za epilogue trigger.
GJ = [2, 2, 4, 8, 8, 8, 12, 12, 4, 2, 2]
assert sum(GJ) == JCH
assert 32 in [sum(GJ[: i + 1]) for i in range(len(GJ))]
GOFF = [sum(GJ[:i]) for i in range(len(GJ))]

# cs upload pieces, alternating rings ahead of the mask groups
CSP = [4, 12, 16, 32]
assert sum(CSP) == JCH
CSOFF = [sum(CSP[:i]) for i in range(len(CSP))]

F8 = mybir.dt.float8e4
F16 = mybir.dt.float16
F32 = mybir.dt.float32

HALF = JCH // 2              # jc < HALF -> Za, else Zb


def build_program() -> bass.Bass:
    nc = bacc.Bacc("TRN2")
    mask_d = nc.dram_tensor("mask", [128, JCH, NISH], F8, kind="ExternalInput")
    cs_d = nc.dram_tensor("cs", [128, JCH, 2 * B], F8, kind="ExternalInput")
    wr_d = nc.dram_tensor("wr", [128, 2, NISH], F8, kind="ExternalInput")
    out_d = nc.dram_tensor("out", [128, 8], F32, kind="ExternalOutput")

    mul = mybir.AluOpType.mult
    rings = [nc.sync, nc.scalar]

    with tile.TileContext(nc) as tc:
        with (
            tc.tile_pool(name="consts", bufs=1) as consts,
            tc.tile_pool(name="masks", bufs=len(GJ)) as masks,
            tc.tile_pool(name="junk", bufs=2) as junkp,
            tc.tile_pool(name="psum", bufs=1, space="PSUM") as psum_pool,
            tc.tile_pool(name="wups", bufs=1, space="PSUM") as wu_pool,
        ):
            # engine-local warm-up operand: no DMA dependency
            wu_sb = consts.tile([128, 512], F16)
            nc.vector.memset(wu_sb[:], 0.0)

            cs_sb = consts.tile([128, JCH, 2 * B], F8)
            rings[0].dma_start(out=cs_sb[:, 0 : CSP[0], :], in_=cs_d[:, 0 : CSP[0], :])
            rings[1].dma_start(
                out=cs_sb[:, CSP[0] : CSOFF[2], :], in_=cs_d[:, CSP[0] : CSOFF[2], :]
            )
            wr_sb = consts.tile([128, 2, NISH], F8)
            racc = consts.tile([128, 8], F32)

            # PE warm-up while the first mask groups are in flight (HAM ramp)
            wu_ps = wu_pool.tile([128, 512], F32)
            for r in range(10):
                nc.tensor.matmul(
                    out=wu_ps[:],
                    lhsT=wu_sb[:, 0:128],
                    rhs=wu_sb[:],
                    start=(r == 0),
                    stop=(r == 9),
                )

            za = psum_pool.tile([128, NISH], F32, tag="za")
            zb = psum_pool.tile([128, NISH], F32, tag="zb")
            zt = [za, zb]

            cs_emitted = 2
            wr_emitted = False
            for g, gj in enumerate(GJ):
                jc0 = GOFF[g]
                ring = rings[g % 2]
                mt = masks.tile([128, gj, NISH], F8, tag="mask")
                ring.dma_start(out=mt[:], in_=mask_d[:, jc0 : jc0 + gj, :])
                if not wr_emitted and jc0 + gj >= 8:
                    # wr needed first by the za epilogue (after jc 31)
                    rings[(g + 1) % 2].dma_start(out=wr_sb[:], in_=wr_d[:])
                    wr_emitted = True
                while cs_emitted < len(CSP) and CSOFF[cs_emitted] < jc0 + gj + 8:
                    p0, pw = CSOFF[cs_emitted], CSP[cs_emitted]
                    rings[(g + 1) % 2].dma_start(
                        out=cs_sb[:, p0 : p0 + pw, :], in_=cs_d[:, p0 : p0 + pw, :]
                    )
                    cs_emitted += 1

                last = g == len(GJ) - 1
                if not last:
                    for k in range(gj):
                        jc = jc0 + k
                        z = zt[jc // HALF]
                        jl = jc % HALF
                        for i0 in range(0, NISH, 512):
                            nc.tensor.matmul(
                                out=z[:, i0 : i0 + 512],
                                lhsT=cs_sb[:, jc, :],
                                rhs=mt[:, k, i0 : i0 + 512],
                                start=(jl == 0),
                                stop=(jl == HALF - 1),
                            )
                else:
                    # last group: i-outer so zb's first half completes early
                    # and its epilogue overlaps the second half's matmuls
                    for ih, i0 in enumerate((0, 512)):
                        for k in range(gj):
                            jc = jc0 + k
                            nc.tensor.matmul(
                                out=zb[:, i0 : i0 + 512],
                                lhsT=cs_sb[:, jc, :],
                                rhs=mt[:, k, i0 : i0 + 512],
                                start=False,
                                stop=(jc == JCH - 1),
                            )
                        isl = slice(i0, i0 + 512)
                        for q in range(2):
                            jr = junkp.tile([128, 512], F16, tag="junk")
                            nc.vector.scalar_tensor_tensor(
                                out=jr[:], in0=zb[:, isl], scalar=1.0,
                                in1=wr_sb[:, q, isl], op0=mul, op1=mul,
                                accum_out=racc[:, 2 + 2 * ih + q : 3 + 2 * ih + q],
                            )
                        ring2 = nc.scalar if ih else nc.sync
                        ring2.dma_start(
                            out=out_d[:, 2 + 2 * ih : 4 + 2 * ih],
                            in_=racc[:, 2 + 2 * ih : 4 + 2 * ih],
                        )

                if jc0 + gj == HALF:
                    # Za complete: epilogue overlaps Zb matmuls
                    for q in range(2):
                        jr = junkp.tile([128, NISH], F16, tag="junk")
                        nc.vector.scalar_tensor_tensor(
                            out=jr[:], in0=za[:], scalar=1.0,
                            in1=wr_sb[:, q, :], op0=mul, op1=mul,
                            accum_out=racc[:, q : q + 1],
                        )
                    nc.sync.dma_start(out=out_d[:, 0:2], in_=racc[:, 0:2])
    nc.finalize()
    return nc


def prep_inputs(phases_a, phases_b, coupling_mask):
    pa = np.asarray(phases_a, dtype=np.float32)
    pb = np.asarray(phases_b, dtype=np.float32)
    ca, sa = np.cos(pa), np.sin(pa)   # (B, NA)
    cb, sb = np.cos(pb), np.sin(pb)   # (B, NB)

    f8np = mybir.dt.np(F8)
    one_byte = np.array([1.0], f8np).view(np.uint8)[0]
    mask_u8 = (np.asarray(coupling_mask) != 0).astype(np.uint8) * one_byte

    # cs[p, jc, m] = (cb|sb)[m, 128*jc + p] — shared by all cores
    csf = np.concatenate([cb, sb], axis=0).astype(f8np)          # (128, NB)
    cs_host = np.ascontiguousarray(
        csf.T.reshape(JCH, 128, 2 * B).transpose(1, 0, 2)
    )

    in_maps = []
    for c in range(NCORES):
        rows = slice(c * NISH, (c + 1) * NISH)
        # mask[p, jc, i] = M[rows[i], 128*jc + p]
        mt = np.ascontiguousarray(
            mask_u8[rows].T.reshape(JCH, 128, NISH).transpose(1, 0, 2)
        ).view(f8np)
        wr = np.empty((128, 2, NISH), np.float32)
        wr[:B, 0] = ca[:, rows]
        wr[B:, 0] = sa[:, rows]
        wr[:B, 1] = sa[:, rows]
        wr[B:, 1] = -ca[:, rows]
        in_maps.append({"mask": mt, "cs": cs_host, "wr": wr.astype(f8np)})
    return in_maps


def combine(outs, coupling_mask):
    o = np.stack(outs).astype(np.float64)          # [NCORES, 128, 8]
    r = (o[:, :, 0] + o[:, :, 2] + o[:, :, 4]).sum(axis=0)   # [128]
    q = (o[:, :, 1] + o[:, :, 3] + o[:, :, 5]).sum(axis=0)
    real = r[:B] + r[B:]
    imag = q[:B] + q[B:]
    n_pairs = max(float(np.asarray(coupling_mask).sum()), 1.0)
    return (np.sqrt(real * real + imag * imag) / n_pairs).astype(np.float32)


_prog_cache: list = []


def kernel(phases_a, phases_b, coupling_mask):
    in_maps = prep_inputs(phases_a, phases_b, coupling_mask)
    if not _prog_cache:
        _prog_cache.append(build_program())
    res = run_bass_kernel_spmd(_prog_cache[0], in_maps, core_ids=list(range(NCORES)))
    return combine([r["out"] for r in res.results], coupling_mask)
